# revision 59
# baseline (speedup 1.0000x reference)
"""DeepSets (MLP + ragged segment-mean) Trainium2 Bass kernel.

Full inputs in / full outputs out. Internally: data-parallel over sets --
tokens are sharded by contiguous whole-segment ranges across 8 NeuronCores
(balanced by token count), the tiny MLP weights are replicated, and the
segment-mean is fully local per core.

v3 design (per-core):
  - x and weights in bf16: halves the dominant HBM stream (matmul rate on
    TRN2 is 1 col/cycle for both bf16 and fp32r, so only DMA gains).
  - L1/L2 matmuls feature-major (weights stationary), fp32 PSUM.
  - PSUM evacuations (bias+relu, cast to bf16) go to DVE + ACT only
    (GPSIMD/Pool cannot touch PSUM); h2 evacs are paired across two ITERs
    into [128,1024] two-bank ops (b2 is one bias vector, so pairing is
    legal; b1 halves differ so h1 stays per-ITER).
  - Cross-window CARRIED cumsum runs on the Pool engine (sbuf->sbuf):
    gathered segment-end cumsums are globally monotone so a plain adjacent
    diff yields segment sums.
  - GpSimd ap_gather picks cumsum at host-computed segment-end columns,
    deferred by one window to avoid head-of-line stalls.
  - Output leaves the device feature-major [128, slots]; the host does the
    transpose and the 1/count scaling (no on-device transpose/scale).
"""

import math
from contextlib import ExitStack

import numpy as np

import concourse.bass as bass
import concourse.tile as tile
from concourse import bacc, mybir
from concourse.bass_utils import run_bass_kernel_spmd

N_CORES = 8
D_IN, D_H, D_OUT = 128, 256, 128
WIN = 2048  # tokens per input-DMA window
SUB = 1024  # tokens per scan/gather sub-window (= one h2 evac pair)
ITER = 512  # tokens per MLP pipeline iteration (= one fp32 psum bank)
SBUF_BUFS = 3

F32 = mybir.dt.float32
BF16 = mybir.dt.bfloat16
I16 = mybir.dt.int16
RELU = mybir.ActivationFunctionType.Relu
ADD = mybir.AluOpType.add
SUBT = mybir.AluOpType.subtract
MULT = mybir.AluOpType.mult
MAX = mybir.AluOpType.max

# static-schedule costs (ns) for psum evacs by engine and free size,
# calibrated against TimelineSim engine-busy traces
_COST = {
    ("dve", 512): 700.0,
    ("act", 512): 615.0,
    ("dve", 1024): 1260.0,
    ("act", 1024): 1070.0,
}


def _build_program(t_pad: int, spw: int, n_tr: int, reps: int = 1, mode: str = "full"):
    """Build the single-core SPMD program for t_pad tokens per core.

    spw: gather slots per window (multiple of 16)
    n_tr: number of 128-slot output tiles (out cols = n_tr*128)
    reps: execute the whole pipeline this many times (timing use only)
    mode: "full" | "dma" | "mm" | "mlp" | "scan" -- ablation timing only
    """
    n_sub = t_pad // SUB
    spw16 = spw // 16
    idxp = ((spw16 + 7) // 8) * 8
    g_len = n_tr * 128

    nc = bacc.Bacc(
        "TRN2", target_bir_lowering=False, debug=False, num_devices=N_CORES
    )
    xT = nc.dram_tensor("xT", [D_IN, t_pad], BF16, kind="ExternalInput").ap()
    w1 = nc.dram_tensor("w1", [D_IN, D_H], BF16, kind="ExternalInput").ap()
    # w2 packed on host: [:, 0:128] = W2[0:128,:], [:, 128:256] = W2[128:256,:]
    w2 = nc.dram_tensor("w2", [128, 2 * D_OUT], BF16, kind="ExternalInput").ap()
    b1 = nc.dram_tensor("b1", [128, 2], F32, kind="ExternalInput").ap()
    b2 = nc.dram_tensor("b2", [128, 1], F32, kind="ExternalInput").ap()
    gidx = nc.dram_tensor("gidx", [128, n_sub * idxp], I16, kind="ExternalInput").ap()
    out = nc.dram_tensor("out", [128, g_len], F32, kind="ExternalOutput").ap()
    # per-sub-window totals: the host adds W_{s-1} to each block-first
    # segment (local scans lose the cross-boundary carry)
    wout = nc.dram_tensor("wout", [128, n_sub], F32, kind="ExternalOutput").ap()

    eng_busy = {"dve": 0.0, "act": 0.0}

    with tile.TileContext(nc) as tc, ExitStack() as ctx:
        singles = ctx.enter_context(tc.tile_pool(name="singles", bufs=1))
        xin = ctx.enter_context(tc.tile_pool(name="xin", bufs=SBUF_BUFS))
        h1sb = ctx.enter_context(tc.tile_pool(name="h1sb", bufs=SBUF_BUFS))
        h2winp = ctx.enter_context(tc.tile_pool(name="h2win", bufs=3))
        winp = ctx.enter_context(tc.tile_pool(name="winp", bufs=3))
        gp = ctx.enter_context(tc.tile_pool(name="gp", bufs=1))
        outp = ctx.enter_context(tc.tile_pool(name="outp", bufs=2))
        # one shared psum ring: h1a/h1b/h2 pair-tiles [128,1024] rotate
        # through 4 slots x 2 banks = all 8 banks; a single tag makes the
        # ring shared so every evacuation is a 1024-element op
        psA = ctx.enter_context(tc.tile_pool(name="psA", bufs=4, space="PSUM"))

        # constants ride the scalar-engine DMA queue so the first input
        # window can issue immediately on the sync queue
        w1s = singles.tile([128, D_H], BF16)
        nc.scalar.dma_start(out=w1s[:], in_=w1[:])
        w2s = singles.tile([128, 2 * D_OUT], BF16)
        nc.scalar.dma_start(out=w2s[:], in_=w2[:])
        b1s = singles.tile([128, 2], F32)
        nc.scalar.dma_start(out=b1s[:], in_=b1[:])
        b2s = singles.tile([128, 1], F32)
        nc.scalar.dma_start(out=b2s[:], in_=b2[:])
        gis = singles.tile([128, n_sub * idxp], I16)
        nc.scalar.dma_start(out=gis[:], in_=gidx[:])
        ones = singles.tile([128, SUB], BF16)
        nc.vector.memset(ones[:], 1.0)
        zcol = singles.tile([128, 1], F32)
        nc.vector.memset(zcol[:], 0.0)

        gpt = gp.tile([128, 1 + g_len], F32, tag="gpad")
        nc.gpsimd.memset(gpt[:], 0.0)
        # touch the activation table at t=0 so the 1.3us table load hides
        # under the first input DMA instead of stalling the first evac
        actwarm = singles.tile([128, 1], F32)
        nc.scalar.activation(actwarm[:], ones[:, 0:2].bitcast(F32), RELU, bias=0.0)

        def evac(dst, src, bias_ap, free):
            """relu(src + bias) -> dst on the less-busy of DVE/ACT."""
            e = min(("dve", "act"), key=lambda k: eng_busy[k] + _COST[(k, free)])
            eng_busy[e] += _COST[(e, free)]
            if e == "act":
                nc.scalar.activation(dst, src, RELU, bias=bias_ap)
            else:
                nc.vector.tensor_scalar(
                    out=dst, in0=src, scalar1=bias_ap, scalar2=0.0, op0=ADD, op1=MAX
                )

        def emit_gather(s, win_s):
            nc.gpsimd.ap_gather(
                out_ap=gpt[:, 1 + s * spw : 1 + (s + 1) * spw],
                in_ap=win_s[:],
                idxs_ap=gis[:, s * idxp : s * idxp + spw16],
                channels=128,
                num_elems=SUB + 1,
                d=1,
                num_idxs=spw,
            )
            # export this sub-window's total for the host-side carry fix
            nc.sync.dma_start(out=wout[:, s : s + 1], in_=win_s[:, SUB : SUB + 1])

        def emit_diff(lo, hi):
            """totals[lo:hi] = gpt[1+lo:1+hi] - gpt[lo:hi], then add the
            previous sub-window's total at each sub-window-boundary slot
            (scans are sub-window-local, so cross-boundary diffs lose the
            carry), then DMA out via the Pool queue (the sync queue carries
            the input stream; a diff-gated store would block it)."""
            n = hi - lo
            tt = outp.tile([128, 1024], F32, tag="tot")
            nc.vector.tensor_tensor(
                out=tt[:, 0:n], in0=gpt[:, 1 + lo : 1 + hi],
                in1=gpt[:, lo:hi], op=SUBT,
            )
            eng_busy["dve"] += (58 + n) * 1.04
            nc.gpsimd.dma_start(out=out[:, lo:hi], in_=tt[:, 0:n])

        for _rep in range(reps):
          # timing-only outer repetition; each rep rewrites the same output
          st = {"prev_win": None, "diffed": 0, "gathered_s": -1}
          pend = []  # [(s, win_tile)] scanned sub-windows, gather deferred
          h2q = []  # [(s, pc_tile)] pairs whose h2 evac+scan is deferred

          def flush_h2():
              if not h2q:
                  return
              s, pc = h2q.pop(0)
              if mode == "full" and pend:
                  # gather for the sub-window scanned one flush ago
                  gs, gwin = pend.pop(0)
                  emit_gather(gs, gwin)
                  st["gathered_s"] = gs
                  safe = st["gathered_s"] * spw
                  while safe - st["diffed"] >= 1024:
                      emit_diff(st["diffed"], st["diffed"] + 1024)
                      st["diffed"] += 1024
              h2w = h2winp.tile([128, SUB], BF16, tag="h2w", name="h2w")
              # pinned to ACT: if this evac ran on DVE, the scan right
              # after would read its output back-to-back on the same
              # engine and pay the full pipeline drain
              nc.scalar.activation(h2w[:], pc[:], RELU, bias=b2s[:, 0:1])
              eng_busy["act"] += _COST[("act", 1024)]
              # sub-window-LOCAL cumsum on DVE (initial = constant zero):
              # local scans are mutually independent, so the hardware never
              # serializes on a scan->scan carry chain (a carried scan costs
              # ~2.1us/scan extra on HW from per-dependent-op pipe drains);
              # the missing carry is restored at diff time from wsums
              win = winp.tile([128, 1 + SUB], F32, tag="win", name="win")
              if s == 0:
                  nc.vector.memset(win[:, 0:1], 0.0)
              nc.vector.tensor_tensor_scan(
                  out=win[:, 1 : 1 + SUB],
                  data0=ones[:],
                  data1=h2w[:],
                  initial=zcol[:],
                  op0=MULT,
                  op1=ADD,
              )
              eng_busy["dve"] += 1190.0
              if mode != "scan":
                  pend.append((s, win))

          # full DMA windows plus an optional SUB-sized tail window
          win_ofs = list(range(0, t_pad - WIN + 1, WIN))
          if t_pad % WIN:
              win_ofs.append(t_pad - SUB)
          for w, ofs in enumerate(win_ofs):
            wlen = WIN if ofs + WIN <= t_pad else SUB
            # one big input DMA per window (4KB per partition in bf16);
            # window 0 is split per-ITER so the pipeline ramps sooner
            xw = xin.tile([128, WIN], BF16, tag="xw")
            if w == 0:
                for j in range(wlen // ITER):
                    nc.sync.dma_start(
                        out=xw[:, j * ITER : (j + 1) * ITER],
                        in_=xT[:, ofs + j * ITER : ofs + (j + 1) * ITER],
                    )
            else:
                nc.sync.dma_start(
                    out=xw[:, 0:wlen], in_=xT[:, ofs : ofs + wlen]
                )
            if mode == "dma":
                nc.vector.tensor_copy(out=gpt[:, 0:1], in_=xw[:, 0:2].bitcast(F32))
                continue
            for p2 in range(wlen // SUB):
                base = p2 * SUB
                s = (ofs + base) // SUB  # sub-window index
                xh = (xw[:, base : base + ITER], xw[:, base + ITER : base + SUB])
                pa = psA.tile([128, SUB], F32, tag="ps", name="h1a_ps")
                pb = psA.tile([128, SUB], F32, tag="ps", name="h1b_ps")
                # L1: one stationary load per weight half per 1024 tokens
                nc.tensor.matmul(pa[:, 0:ITER], w1s[:, 0:128], xh[0],
                                 start=True, stop=True)
                nc.tensor.matmul(pa[:, ITER:SUB], w1s[:, 0:128], xh[1],
                                 start=True, stop=True)
                nc.tensor.matmul(pb[:, 0:ITER], w1s[:, 128:256], xh[0],
                                 start=True, stop=True)
                nc.tensor.matmul(pb[:, ITER:SUB], w1s[:, 128:256], xh[1],
                                 start=True, stop=True)
                if mode == "mm":
                    nc.vector.tensor_copy(out=gpt[:, 0:1], in_=pa[:, 0:1])
                    nc.vector.tensor_copy(out=gpt[:, 0:1], in_=pb[:, 0:1])
                    continue
                h1a = h1sb.tile([128, SUB], BF16, tag="h1a")
                h1b = h1sb.tile([128, SUB], BF16, tag="h1b")
                # W2a matmuls are ordered right after the h1a evacuation so
                # the PE doesn't also wait on h1b's evacuation
                evac(h1a[:], pa[:], b1s[:, 0:1], 1024)
                pc = psA.tile([128, SUB], F32, tag="ps", name="h2_ps")
                nc.tensor.matmul(pc[:, 0:ITER], w2s[:, 0:128], h1a[:, 0:ITER],
                                 start=True, stop=False)
                nc.tensor.matmul(pc[:, ITER:SUB], w2s[:, 0:128], h1a[:, ITER:SUB],
                                 start=True, stop=False)
                evac(h1b[:], pb[:], b1s[:, 1:2], 1024)
                nc.tensor.matmul(pc[:, 0:ITER], w2s[:, 128:256], h1b[:, 0:ITER],
                                 start=False, stop=True)
                nc.tensor.matmul(pc[:, ITER:SUB], w2s[:, 128:256], h1b[:, ITER:SUB],
                                 start=False, stop=True)
                if mode == "mlp":
                    nc.vector.tensor_copy(out=gpt[:, 0:1], in_=pc[:, 0:1])
                    continue
                h2q.append((s, pc))
                flush_h2()
          if mode in ("full", "scan"):
            flush_h2()
          if mode == "full":
            total_slots = n_sub * spw
            # diffs that depend only on already-emitted gathers go first so
            # only the last spw slots wait on the final gather
            while pend:
                safe = pend[0][0] * spw
                while safe - st["diffed"] >= 1 and st["diffed"] < safe:
                    take = min(1024, safe - st["diffed"])
                    emit_diff(st["diffed"], st["diffed"] + take)
                    st["diffed"] += take
                gs, gwin = pend.pop(0)
                emit_gather(gs, gwin)
            while st["diffed"] < total_slots:
                take = min(1024, total_slots - st["diffed"])
                emit_diff(st["diffed"], st["diffed"] + take)
                st["diffed"] += take

    nc.compile()
    return nc


def _prepare(x, segment_ids, num_segments):
    """Host-side sharding + gather-index construction. Returns per-core
    metadata and the program size parameters."""
    T_total = x.shape[0]
    n_seg = int(num_segments)
    seg = np.asarray(segment_ids).astype(np.int64)
    counts = np.bincount(seg, minlength=n_seg).astype(np.int64)
    # local scans + single-carry boundary fixup assume a segment never
    # spans more than two sub-windows
    assert counts.max() < SUB, "segment longer than scan sub-window"
    cum = np.cumsum(counts)

    # whole-segment split balanced by token count
    split = [0]
    for c in range(1, N_CORES):
        target = c * T_total / N_CORES
        s = int(np.searchsorted(cum, target))
        if s + 1 < n_seg and abs(cum[s] - target) < abs(
            (cum[s - 1] if s > 0 else 0) - target
        ):
            s = s + 1
        s = max(split[-1], min(s, n_seg))
        split.append(s)
    split.append(n_seg)

    cores = []
    max_tok = 1
    for c in range(N_CORES):
        s0, s1 = split[c], split[c + 1]
        t0 = int(cum[s0 - 1]) if s0 > 0 else 0
        t1 = int(cum[s1 - 1]) if s1 > 0 else 0
        cores.append({"s0": s0, "s1": s1, "t0": t0, "t1": t1})
        max_tok = max(max_tok, t1 - t0)

    # pad to SUB granularity (not WIN): the DMA loop handles a SUB-sized
    # tail window, and a whole mostly-pad sub-window is avoided
    t_pad = int(math.ceil(max_tok / SUB) * SUB)
    n_sub = t_pad // SUB

    # per-core per-sub-window segment-end indices
    max_ends = 1
    for core in cores:
        s0, s1, t0 = core["s0"], core["s1"], core["t0"]
        ends = cum[s0:s1] - 1 - t0  # local end col per segment; may be -1
        sub_of = np.maximum(ends, 0) // SUB
        idx_rel = ends - sub_of * SUB + 1  # in [0, SUB]
        core["sub_of"] = sub_of
        core["idx_rel"] = idx_rel
        if len(ends):
            bc = np.bincount(sub_of, minlength=n_sub)
            max_ends = max(max_ends, int(bc.max()))

    spw = int(math.ceil(max_ends / 16) * 16)
    n_tr = int(math.ceil(n_sub * spw / 128))

    for core in cores:
        s0, s1 = core["s0"], core["s1"]
        n_loc = s1 - s0
        slot_of = np.zeros(n_loc, dtype=np.int64)
        idx_full = np.zeros(n_sub * spw, dtype=np.int16)
        pos = np.zeros(n_sub, dtype=np.int64)
        # fill sub-window-by-sub-window in segment order
        for j in range(n_loc):
            w = int(core["sub_of"][j])
            k = int(pos[w])
            assert k < spw
            idx_full[w * spw + k] = core["idx_rel"][j]
            slot_of[j] = w * spw + k
            pos[w] = k + 1
        # pad each sub-window by repeating its last real index (0 if none)
        for w in range(n_sub):
            k = int(pos[w])
            last = idx_full[w * spw + k - 1] if k > 0 else np.int16(0)
            idx_full[w * spw + k : (w + 1) * spw] = last
        core["slot_of"] = slot_of
        # wrap for ap_gather: unwrapped[j] = idxs[j % 16, j // 16] per
        # sub-window, each block padded to a 16B-aligned width
        idxp = ((spw // 16 + 7) // 8) * 8
        blocks = []
        for w in range(n_sub):
            arr = idx_full[w * spw : (w + 1) * spw]
            blk = np.zeros((16, idxp), dtype=np.int16)
            blk[:, : spw // 16] = arr.reshape(spw // 16, 16).T
            blocks.append(blk)
        gidx16 = np.concatenate(blocks, axis=1)  # [16, n_sub * idxp]
        core["gidx"] = np.tile(gidx16, (8, 1)).astype(np.int16)  # [128, ...]
        core["inv"] = (1.0 / np.maximum(counts[s0:s1], 1)).astype(np.float32)

    return cores, t_pad, spw, n_tr


_PROGRAM_CACHE = {}


def _bf16(a):
    import ml_dtypes

    return np.asarray(a, dtype=np.float32).astype(ml_dtypes.bfloat16)


def _make_in_maps(cores, t_pad, x, W1, b1, W2, b2):
    w1_np = _bf16(W1)
    w2_np = _bf16(np.concatenate([W2[:128, :], W2[128:, :]], axis=1))
    b1_np = np.ascontiguousarray(np.stack([b1[:128], b1[128:]], axis=1))
    b2_np = np.ascontiguousarray(b2[:, None])
    in_maps = []
    for core in cores:
        t0, t1 = core["t0"], core["t1"]
        xT_c = np.zeros((D_IN, t_pad), dtype=np.float32)
        xT_c[:, : t1 - t0] = x[t0:t1].T
        in_maps.append(
            {
                "xT": _bf16(xT_c),
                "w1": w1_np,
                "w2": w2_np,
                "b1": b1_np,
                "b2": b2_np,
                "gidx": core["gidx"],
            }
        )
    return in_maps


def kernel(x, segment_ids, num_segments, W1, b1, W2, b2):
    x = np.ascontiguousarray(np.asarray(x, dtype=np.float32))
    W1 = np.asarray(W1, dtype=np.float32)
    b1 = np.asarray(b1, dtype=np.float32)
    W2 = np.asarray(W2, dtype=np.float32)
    b2 = np.asarray(b2, dtype=np.float32)
    n_seg = int(num_segments)

    cores, t_pad, spw, n_tr = _prepare(x, segment_ids, num_segments)

    key = (t_pad, spw, n_tr)
    if key not in _PROGRAM_CACHE:
        _PROGRAM_CACHE[key] = _build_program(t_pad, spw, n_tr)
    nc = _PROGRAM_CACHE[key]

    in_maps = _make_in_maps(cores, t_pad, x, W1, b1, W2, b2)

    res = run_bass_kernel_spmd(nc, in_maps, list(range(N_CORES)))

    out_full = np.zeros((n_seg, D_OUT), dtype=np.float32)
    for c, core in enumerate(cores):
        s0, s1 = core["s0"], core["s1"]
        if s1 > s0:
            vals = res.results[c]["out"][:, core["slot_of"]]  # [128, n_loc]
            out_full[s0:s1] = (vals * core["inv"][None, :]).T
    return out_full


# revision 68
# speedup vs baseline: 1.4205x; 1.4205x over previous
"""DeepSets (MLP + ragged segment-mean) Trainium2 Bass kernel.

Full inputs in / full outputs out. Internally: data-parallel over sets --
tokens are sharded by contiguous whole-segment ranges across 8 NeuronCores
(balanced by token count), the tiny MLP weights are replicated, and the
segment-mean is fully local per core.

v3 design (per-core):
  - x and weights in bf16: halves the dominant HBM stream (matmul rate on
    TRN2 is 1 col/cycle for both bf16 and fp32r, so only DMA gains).
  - L1/L2 matmuls feature-major (weights stationary), fp32 PSUM.
  - PSUM evacuations (bias+relu, cast to bf16) go to DVE + ACT only
    (GPSIMD/Pool cannot touch PSUM); h2 evacs are paired across two ITERs
    into [128,1024] two-bank ops (b2 is one bias vector, so pairing is
    legal; b1 halves differ so h1 stays per-ITER).
  - Cross-window CARRIED cumsum runs on the Pool engine (sbuf->sbuf):
    gathered segment-end cumsums are globally monotone so a plain adjacent
    diff yields segment sums.
  - GpSimd ap_gather picks cumsum at host-computed segment-end columns,
    deferred by one window to avoid head-of-line stalls.
  - Output leaves the device feature-major [128, slots]; the host does the
    transpose and the 1/count scaling (no on-device transpose/scale).
"""

import math
from contextlib import ExitStack

import numpy as np

import concourse.bass as bass
import concourse.tile as tile
from concourse import bacc, mybir
from concourse.bass_utils import run_bass_kernel_spmd

N_CORES = 8
D_IN, D_H, D_OUT = 128, 256, 128
WIN = 2048  # tokens per input-DMA window
SUB = 1024  # tokens per scan/gather sub-window (= one h2 evac pair)
SUB2 = SUB // 2  # pair-sums per sub-window (segments are padded to even
#                  length so every segment boundary falls between pairs)
ITER = 512  # tokens per MLP pipeline iteration (= one fp32 psum bank)
SBUF_BUFS = 3

F32 = mybir.dt.float32
BF16 = mybir.dt.bfloat16
I16 = mybir.dt.int16
RELU = mybir.ActivationFunctionType.Relu
ADD = mybir.AluOpType.add
SUBT = mybir.AluOpType.subtract
MULT = mybir.AluOpType.mult
MAX = mybir.AluOpType.max

# static-schedule costs (ns) for psum evacs by engine and free size,
# calibrated against TimelineSim engine-busy traces
_COST = {
    ("dve", 512): 700.0,
    ("act", 512): 615.0,
    ("dve", 1024): 1260.0,
    ("act", 1024): 1070.0,
}


def _build_program(t_pad: int, spw: int, n_tr: int, reps: int = 1, mode: str = "full"):
    """Build the single-core SPMD program for t_pad tokens per core.

    spw: gather slots per window (multiple of 16)
    n_tr: number of 128-slot output tiles (out cols = n_tr*128)
    reps: execute the whole pipeline this many times (timing use only)
    mode: "full" | "dma" | "mm" | "mlp" | "scan" -- ablation timing only
    """
    n_sub = t_pad // SUB
    spw16 = spw // 16
    idxp = ((spw16 + 7) // 8) * 8
    g_len = n_tr * 128

    nc = bacc.Bacc(
        "TRN2", target_bir_lowering=False, debug=False, num_devices=N_CORES
    )
    xT = nc.dram_tensor("xT", [D_IN, t_pad], BF16, kind="ExternalInput").ap()
    w1 = nc.dram_tensor("w1", [D_IN, D_H], BF16, kind="ExternalInput").ap()
    # w2 packed on host: [:, 0:128] = W2[0:128,:], [:, 128:256] = W2[128:256,:]
    w2 = nc.dram_tensor("w2", [128, 2 * D_OUT], BF16, kind="ExternalInput").ap()
    b1 = nc.dram_tensor("b1", [128, 2], F32, kind="ExternalInput").ap()
    b2 = nc.dram_tensor("b2", [128, 1], F32, kind="ExternalInput").ap()
    gidx = nc.dram_tensor("gidx", [128, n_sub * idxp], I16, kind="ExternalInput").ap()
    out = nc.dram_tensor("out", [128, g_len], F32, kind="ExternalOutput").ap()
    # per-sub-window totals: the host adds W_{s-1} to each block-first
    # segment (local scans lose the cross-boundary carry)
    wout = nc.dram_tensor("wout", [128, n_sub], F32, kind="ExternalOutput").ap()

    eng_busy = {"dve": 0.0, "act": 0.0}

    with tile.TileContext(nc) as tc, ExitStack() as ctx:
        singles = ctx.enter_context(tc.tile_pool(name="singles", bufs=1))
        xin = ctx.enter_context(tc.tile_pool(name="xin", bufs=SBUF_BUFS))
        h1sb = ctx.enter_context(tc.tile_pool(name="h1sb", bufs=SBUF_BUFS))
        h2winp = ctx.enter_context(tc.tile_pool(name="h2win", bufs=3))
        winp = ctx.enter_context(tc.tile_pool(name="winp", bufs=3))
        gp = ctx.enter_context(tc.tile_pool(name="gp", bufs=1))
        outp = ctx.enter_context(tc.tile_pool(name="outp", bufs=2))
        # one shared psum ring: h1a/h1b/h2 pair-tiles [128,1024] rotate
        # through 4 slots x 2 banks = all 8 banks; a single tag makes the
        # ring shared so every evacuation is a 1024-element op
        psA = ctx.enter_context(tc.tile_pool(name="psA", bufs=4, space="PSUM"))

        # constants ride the scalar-engine DMA queue so the first input
        # window can issue immediately on the sync queue
        w1s = singles.tile([128, D_H], BF16)
        nc.scalar.dma_start(out=w1s[:], in_=w1[:])
        w2s = singles.tile([128, 2 * D_OUT], BF16)
        nc.scalar.dma_start(out=w2s[:], in_=w2[:])
        b1s = singles.tile([128, 2], F32)
        nc.scalar.dma_start(out=b1s[:], in_=b1[:])
        b2s = singles.tile([128, 1], F32)
        nc.scalar.dma_start(out=b2s[:], in_=b2[:])
        gis = singles.tile([128, n_sub * idxp], I16)
        nc.scalar.dma_start(out=gis[:], in_=gidx[:])
        ones = singles.tile([128, SUB], BF16)
        nc.vector.memset(ones[:], 1.0)
        zcol = singles.tile([128, 1], F32)
        nc.vector.memset(zcol[:], 0.0)

        gpt = gp.tile([128, 1 + g_len], F32, tag="gpad")
        nc.gpsimd.memset(gpt[:], 0.0)
        # touch the activation table at t=0 so the 1.3us table load hides
        # under the first input DMA instead of stalling the first evac
        actwarm = singles.tile([128, 1], F32)
        nc.scalar.activation(actwarm[:], ones[:, 0:2].bitcast(F32), RELU, bias=0.0)

        def evac(dst, src, bias_ap, free):
            """relu(src + bias) -> dst on the less-busy of DVE/ACT."""
            e = min(("dve", "act"), key=lambda k: eng_busy[k] + _COST[(k, free)])
            eng_busy[e] += _COST[(e, free)]
            if e == "act":
                nc.scalar.activation(dst, src, RELU, bias=bias_ap)
            else:
                nc.vector.tensor_scalar(
                    out=dst, in0=src, scalar1=bias_ap, scalar2=0.0, op0=ADD, op1=MAX
                )

        def emit_gather(s, win_s):
            nc.gpsimd.ap_gather(
                out_ap=gpt[:, 1 + s * spw : 1 + (s + 1) * spw],
                in_ap=win_s[:],
                idxs_ap=gis[:, s * idxp : s * idxp + spw16],
                channels=128,
                num_elems=SUB2 + 1,
                d=1,
                num_idxs=spw,
            )
            # export this sub-window's total for the host-side carry fix
            nc.sync.dma_start(
                out=wout[:, s : s + 1], in_=win_s[:, SUB2 : SUB2 + 1]
            )

        def emit_diff(lo, hi):
            """totals[lo:hi] = gpt[1+lo:1+hi] - gpt[lo:hi], then add the
            previous sub-window's total at each sub-window-boundary slot
            (scans are sub-window-local, so cross-boundary diffs lose the
            carry), then DMA out via the Pool queue (the sync queue carries
            the input stream; a diff-gated store would block it)."""
            n = hi - lo
            tt = outp.tile([128, 1024], F32, tag="tot")
            nc.vector.tensor_tensor(
                out=tt[:, 0:n], in0=gpt[:, 1 + lo : 1 + hi],
                in1=gpt[:, lo:hi], op=SUBT,
            )
            eng_busy["dve"] += (58 + n) * 1.04
            nc.gpsimd.dma_start(out=out[:, lo:hi], in_=tt[:, 0:n])

        for _rep in range(reps):
          # timing-only outer repetition; each rep rewrites the same output
          st = {"diffed": 0, "gathered_s": -1}
          pend = []  # [(s, win_tile)] scanned sub-windows, gather deferred
          h2q = []  # [(s, pc_tile)] pairs whose h2 evac is deferred
          p2q = []  # [(s, p2_tile)] pair-sums whose scan is deferred

          def emit_scan(s, p2t):
              # sub-window-LOCAL cumsum over pair-sums (initial = const 0):
              # local scans are mutually independent -- no carry chain; the
              # missing carry is restored on the host from wout. Deferred
              # one pair behind its producer so the DVE never runs the scan
              # directly after the pair-add that feeds it (pipe drain).
              win = winp.tile([128, 1 + SUB2], F32, tag="win", name="win")
              if s == 0:
                  nc.vector.memset(win[:, 0:1], 0.0)
              nc.vector.tensor_tensor_scan(
                  out=win[:, 1 : 1 + SUB2],
                  data0=ones[:, 0:SUB2],
                  data1=p2t[:],
                  initial=zcol[:],
                  op0=MULT,
                  op1=ADD,
              )
              eng_busy["dve"] += 1660.0
              if mode != "scan":
                  pend.append((s, win))

          def flush_h2():
              if not h2q:
                  return
              s, pc = h2q.pop(0)
              if mode == "full" and pend:
                  # gather for the sub-window scanned one flush ago
                  gs, gwin = pend.pop(0)
                  emit_gather(gs, gwin)
                  st["gathered_s"] = gs
                  safe = st["gathered_s"] * spw
                  while safe - st["diffed"] >= 1024:
                      emit_diff(st["diffed"], st["diffed"] + 1024)
                      st["diffed"] += 1024
              h2w = h2winp.tile([128, SUB], BF16, tag="h2w", name="h2w")
              # pinned to ACT: if this evac ran on DVE, the pair-add right
              # after would read its output back-to-back on the same
              # engine and pay the full pipeline drain
              nc.scalar.activation(h2w[:], pc[:], RELU, bias=b2s[:, 0:1])
              eng_busy["act"] += _COST[("act", 1024)]
              # adjacent-pair sums (segments are even-length, so pairs never
              # straddle a boundary): a plain pipelineable add, which halves
              # the length of the scan -- the scan's loop-carried recurrence
              # runs at only ~1 elem / 3 cycles on hardware
              p2t = winp.tile([128, SUB2], F32, tag="p2", name="p2")
              nc.vector.tensor_tensor(
                  out=p2t[:], in0=h2w[:, 0:SUB:2], in1=h2w[:, 1:SUB:2], op=ADD
              )
              eng_busy["dve"] += 660.0
              if p2q:
                  emit_scan(*p2q.pop(0))
              p2q.append((s, p2t))

          # full DMA windows plus an optional SUB-sized tail window
          win_ofs = list(range(0, t_pad - WIN + 1, WIN))
          if t_pad % WIN:
              win_ofs.append(t_pad - SUB)
          for w, ofs in enumerate(win_ofs):
            wlen = WIN if ofs + WIN <= t_pad else SUB
            # one big input DMA per window (4KB per partition in bf16);
            # window 0 is split per-ITER so the pipeline ramps sooner
            xw = xin.tile([128, WIN], BF16, tag="xw")
            if w == 0:
                for j in range(wlen // ITER):
                    nc.sync.dma_start(
                        out=xw[:, j * ITER : (j + 1) * ITER],
                        in_=xT[:, ofs + j * ITER : ofs + (j + 1) * ITER],
                    )
            else:
                nc.sync.dma_start(
                    out=xw[:, 0:wlen], in_=xT[:, ofs : ofs + wlen]
                )
            if mode == "dma":
                nc.vector.tensor_copy(out=gpt[:, 0:1], in_=xw[:, 0:2].bitcast(F32))
                continue
            for p2 in range(wlen // SUB):
                base = p2 * SUB
                s = (ofs + base) // SUB  # sub-window index
                xh = (xw[:, base : base + ITER], xw[:, base + ITER : base + SUB])
                pa = psA.tile([128, SUB], F32, tag="ps", name="h1a_ps")
                pb = psA.tile([128, SUB], F32, tag="ps", name="h1b_ps")
                # L1: one stationary load per weight half per 1024 tokens
                nc.tensor.matmul(pa[:, 0:ITER], w1s[:, 0:128], xh[0],
                                 start=True, stop=True)
                nc.tensor.matmul(pa[:, ITER:SUB], w1s[:, 0:128], xh[1],
                                 start=True, stop=True)
                nc.tensor.matmul(pb[:, 0:ITER], w1s[:, 128:256], xh[0],
                                 start=True, stop=True)
                nc.tensor.matmul(pb[:, ITER:SUB], w1s[:, 128:256], xh[1],
                                 start=True, stop=True)
                if mode == "mm":
                    nc.vector.tensor_copy(out=gpt[:, 0:1], in_=pa[:, 0:1])
                    nc.vector.tensor_copy(out=gpt[:, 0:1], in_=pb[:, 0:1])
                    continue
                h1a = h1sb.tile([128, SUB], BF16, tag="h1a")
                h1b = h1sb.tile([128, SUB], BF16, tag="h1b")
                # W2a matmuls are ordered right after the h1a evacuation so
                # the PE doesn't also wait on h1b's evacuation
                evac(h1a[:], pa[:], b1s[:, 0:1], 1024)
                pc = psA.tile([128, SUB], F32, tag="ps", name="h2_ps")
                nc.tensor.matmul(pc[:, 0:ITER], w2s[:, 0:128], h1a[:, 0:ITER],
                                 start=True, stop=False)
                nc.tensor.matmul(pc[:, ITER:SUB], w2s[:, 0:128], h1a[:, ITER:SUB],
                                 start=True, stop=False)
                evac(h1b[:], pb[:], b1s[:, 1:2], 1024)
                nc.tensor.matmul(pc[:, 0:ITER], w2s[:, 128:256], h1b[:, 0:ITER],
                                 start=False, stop=True)
                nc.tensor.matmul(pc[:, ITER:SUB], w2s[:, 128:256], h1b[:, ITER:SUB],
                                 start=False, stop=True)
                if mode == "mlp":
                    nc.vector.tensor_copy(out=gpt[:, 0:1], in_=pc[:, 0:1])
                    continue
                h2q.append((s, pc))
                flush_h2()
          if mode in ("full", "scan"):
            flush_h2()
            while p2q:
                emit_scan(*p2q.pop(0))
          if mode == "full":
            total_slots = n_sub * spw
            # diffs that depend only on already-emitted gathers go first so
            # only the last spw slots wait on the final gather
            while pend:
                safe = pend[0][0] * spw
                while safe - st["diffed"] >= 1 and st["diffed"] < safe:
                    take = min(1024, safe - st["diffed"])
                    emit_diff(st["diffed"], st["diffed"] + take)
                    st["diffed"] += take
                gs, gwin = pend.pop(0)
                emit_gather(gs, gwin)
            while st["diffed"] < total_slots:
                take = min(1024, total_slots - st["diffed"])
                emit_diff(st["diffed"], st["diffed"] + take)
                st["diffed"] += take

    nc.compile()
    return nc


def _prepare(x, segment_ids, num_segments):
    """Host-side sharding + gather-index construction. Returns per-core
    metadata and the program size parameters."""
    T_total = x.shape[0]
    n_seg = int(num_segments)
    seg = np.asarray(segment_ids).astype(np.int64)
    counts = np.bincount(seg, minlength=n_seg).astype(np.int64)
    # pad every segment to an even token count (zero tokens, corrected on
    # the host) so adjacent-pair sums never straddle a segment boundary
    counts2 = ((counts + 1) // 2) * 2
    pad = counts2 - counts
    # local scans + single-carry boundary fixup assume a segment never
    # spans more than two sub-windows
    assert counts2.max() < SUB, "segment longer than scan sub-window"
    cum = np.cumsum(counts)
    cum2 = np.cumsum(counts2)

    # whole-segment split balanced by token count
    split = [0]
    for c in range(1, N_CORES):
        target = c * T_total / N_CORES
        s = int(np.searchsorted(cum, target))
        if s + 1 < n_seg and abs(cum[s] - target) < abs(
            (cum[s - 1] if s > 0 else 0) - target
        ):
            s = s + 1
        s = max(split[-1], min(s, n_seg))
        split.append(s)
    split.append(n_seg)

    cores = []
    max_tok = 1
    for c in range(N_CORES):
        s0, s1 = split[c], split[c + 1]
        t0 = int(cum[s0 - 1]) if s0 > 0 else 0
        t1 = int(cum[s1 - 1]) if s1 > 0 else 0
        t0p = int(cum2[s0 - 1]) if s0 > 0 else 0
        t1p = int(cum2[s1 - 1]) if s1 > 0 else 0
        pad_loc = pad[s0:s1]
        # dst column (in the padded stream) of each real token
        pads_before = np.concatenate([[0], np.cumsum(pad_loc[:-1])]) \
            if s1 > s0 else np.zeros(0, dtype=np.int64)
        dst_idx = np.arange(t1 - t0) + np.repeat(pads_before, counts[s0:s1])
        cores.append(
            {"s0": s0, "s1": s1, "t0": t0, "t1": t1, "t0p": t0p, "t1p": t1p,
             "pad": pad_loc, "dst_idx": dst_idx}
        )
        max_tok = max(max_tok, t1p - t0p)

    # pad to SUB granularity (not WIN): the DMA loop handles a SUB-sized
    # tail window, and a whole mostly-pad sub-window is avoided
    t_pad = int(math.ceil(max_tok / SUB) * SUB)
    n_sub = t_pad // SUB

    # per-core per-sub-window segment-end indices (in the padded stream;
    # gather indices count token PAIRS)
    max_ends = 1
    for core in cores:
        s0, s1, t0p = core["s0"], core["s1"], core["t0p"]
        ends = cum2[s0:s1] - 1 - t0p  # local end col per segment; may be -1
        sub_of = np.maximum(ends, 0) // SUB
        idx_rel = (ends - sub_of * SUB + 1) // 2  # pair idx in [0, SUB2]
        core["sub_of"] = sub_of
        core["idx_rel"] = idx_rel
        if len(ends):
            bc = np.bincount(sub_of, minlength=n_sub)
            max_ends = max(max_ends, int(bc.max()))

    spw = int(math.ceil(max_ends / 16) * 16)
    n_tr = int(math.ceil(n_sub * spw / 128))

    for core in cores:
        s0, s1 = core["s0"], core["s1"]
        n_loc = s1 - s0
        slot_of = np.zeros(n_loc, dtype=np.int64)
        idx_full = np.zeros(n_sub * spw, dtype=np.int16)
        pos = np.zeros(n_sub, dtype=np.int64)
        # fill sub-window-by-sub-window in segment order
        for j in range(n_loc):
            w = int(core["sub_of"][j])
            k = int(pos[w])
            assert k < spw
            idx_full[w * spw + k] = core["idx_rel"][j]
            slot_of[j] = w * spw + k
            pos[w] = k + 1
        # pad each sub-window by repeating its last real index (0 if none)
        for w in range(n_sub):
            k = int(pos[w])
            last = idx_full[w * spw + k - 1] if k > 0 else np.int16(0)
            idx_full[w * spw + k : (w + 1) * spw] = last
        core["slot_of"] = slot_of
        # wrap for ap_gather: unwrapped[j] = idxs[j % 16, j // 16] per
        # sub-window, each block padded to a 16B-aligned width
        idxp = ((spw // 16 + 7) // 8) * 8
        blocks = []
        for w in range(n_sub):
            arr = idx_full[w * spw : (w + 1) * spw]
            blk = np.zeros((16, idxp), dtype=np.int16)
            blk[:, : spw // 16] = arr.reshape(spw // 16, 16).T
            blocks.append(blk)
        gidx16 = np.concatenate(blocks, axis=1)  # [16, n_sub * idxp]
        core["gidx"] = np.tile(gidx16, (8, 1)).astype(np.int16)  # [128, ...]
        core["inv"] = (1.0 / np.maximum(counts[s0:s1], 1)).astype(np.float32)

    return cores, t_pad, spw, n_tr


_PROGRAM_CACHE = {}


def _bf16(a):
    import ml_dtypes

    return np.asarray(a, dtype=np.float32).astype(ml_dtypes.bfloat16)


def _make_in_maps(cores, t_pad, x, W1, b1, W2, b2):
    w1_np = _bf16(W1)
    w2_np = _bf16(np.concatenate([W2[:128, :], W2[128:, :]], axis=1))
    b1_np = np.ascontiguousarray(np.stack([b1[:128], b1[128:]], axis=1))
    b2_np = np.ascontiguousarray(b2[:, None])
    in_maps = []
    for core in cores:
        t0, t1 = core["t0"], core["t1"]
        xT_c = np.zeros((D_IN, t_pad), dtype=np.float32)
        xT_c[:, core["dst_idx"]] = x[t0:t1].T
        in_maps.append(
            {
                "xT": _bf16(xT_c),
                "w1": w1_np,
                "w2": w2_np,
                "b1": b1_np,
                "b2": b2_np,
                "gidx": core["gidx"],
            }
        )
    return in_maps


def kernel(x, segment_ids, num_segments, W1, b1, W2, b2):
    x = np.ascontiguousarray(np.asarray(x, dtype=np.float32))
    W1 = np.asarray(W1, dtype=np.float32)
    b1 = np.asarray(b1, dtype=np.float32)
    W2 = np.asarray(W2, dtype=np.float32)
    b2 = np.asarray(b2, dtype=np.float32)
    n_seg = int(num_segments)

    cores, t_pad, spw, n_tr = _prepare(x, segment_ids, num_segments)

    key = (t_pad, spw, n_tr)
    if key not in _PROGRAM_CACHE:
        _PROGRAM_CACHE[key] = _build_program(t_pad, spw, n_tr)
    nc = _PROGRAM_CACHE[key]

    in_maps = _make_in_maps(cores, t_pad, x, W1, b1, W2, b2)

    res = run_bass_kernel_spmd(nc, in_maps, list(range(N_CORES)))

    # constant h2 contribution of a zero pad token, in device arithmetic
    import ml_dtypes

    h1p = _bf16(np.maximum(b1, 0)).astype(np.float32)
    w2b = _bf16(W2).astype(np.float32)
    cpad = _bf16(np.maximum(h1p @ w2b + b2, 0)).astype(np.float32)  # [128]

    out_full = np.zeros((n_seg, D_OUT), dtype=np.float32)
    for c, core in enumerate(cores):
        s0, s1 = core["s0"], core["s1"]
        if s1 <= s0:
            continue
        slot = core["slot_of"]
        vals = res.results[c]["out"][:, slot]  # [128, n_loc] segment sums
        # cross-sub-window carry: block-first segments add the previous
        # sub-window's total (scans are sub-window-local)
        wout = res.results[c]["wout"]  # [128, n_sub]
        first = (slot % spw == 0) & (slot >= spw)
        if first.any():
            vals[:, first] += wout[:, slot[first] // spw - 1]
        # remove the pad tokens' constant contribution, then mean
        vals = vals - cpad[:, None] * core["pad"][None, :]
        out_full[s0:s1] = (vals * core["inv"][None, :]).T
    return out_full


# revision 69
# speedup vs baseline: 1.4638x; 1.0305x over previous
"""DeepSets (MLP + ragged segment-mean) Trainium2 Bass kernel.

Full inputs in / full outputs out. Internally: data-parallel over sets --
tokens are sharded by contiguous whole-segment ranges across 8 NeuronCores
(balanced by token count), the tiny MLP weights are replicated, and the
segment-mean is fully local per core.

v3 design (per-core):
  - x and weights in bf16: halves the dominant HBM stream (matmul rate on
    TRN2 is 1 col/cycle for both bf16 and fp32r, so only DMA gains).
  - L1/L2 matmuls feature-major (weights stationary), fp32 PSUM.
  - PSUM evacuations (bias+relu, cast to bf16) go to DVE + ACT only
    (GPSIMD/Pool cannot touch PSUM); h2 evacs are paired across two ITERs
    into [128,1024] two-bank ops (b2 is one bias vector, so pairing is
    legal; b1 halves differ so h1 stays per-ITER).
  - Cross-window CARRIED cumsum runs on the Pool engine (sbuf->sbuf):
    gathered segment-end cumsums are globally monotone so a plain adjacent
    diff yields segment sums.
  - GpSimd ap_gather picks cumsum at host-computed segment-end columns,
    deferred by one window to avoid head-of-line stalls.
  - Output leaves the device feature-major [128, slots]; the host does the
    transpose and the 1/count scaling (no on-device transpose/scale).
"""

import math
from contextlib import ExitStack

import numpy as np

import concourse.bass as bass
import concourse.tile as tile
from concourse import bacc, mybir
from concourse.bass_utils import run_bass_kernel_spmd

N_CORES = 8
D_IN, D_H, D_OUT = 128, 256, 128
WIN = 2048  # tokens per input-DMA window
SUB = 1024  # tokens per scan/gather sub-window (= one h2 evac pair)
SUB2 = SUB // 2  # pair-sums per sub-window (segments are padded to even
#                  length so every segment boundary falls between pairs)
ITER = 512  # tokens per MLP pipeline iteration (= one fp32 psum bank)
SBUF_BUFS = 3

F32 = mybir.dt.float32
BF16 = mybir.dt.bfloat16
I16 = mybir.dt.int16
RELU = mybir.ActivationFunctionType.Relu
ADD = mybir.AluOpType.add
SUBT = mybir.AluOpType.subtract
MULT = mybir.AluOpType.mult
MAX = mybir.AluOpType.max

# static-schedule costs (ns) for psum evacs by engine and free size,
# calibrated against TimelineSim engine-busy traces
_COST = {
    ("dve", 512): 700.0,
    ("act", 512): 615.0,
    ("dve", 1024): 1260.0,
    ("act", 1024): 1070.0,
}


def _build_program(t_pad: int, spw: int, n_tr: int, reps: int = 1, mode: str = "full"):
    """Build the single-core SPMD program for t_pad tokens per core.

    spw: gather slots per window (multiple of 16)
    n_tr: number of 128-slot output tiles (out cols = n_tr*128)
    reps: execute the whole pipeline this many times (timing use only)
    mode: "full" | "dma" | "mm" | "mlp" | "scan" -- ablation timing only
    """
    n_sub = t_pad // SUB
    spw16 = spw // 16
    idxp = ((spw16 + 7) // 8) * 8
    g_len = n_tr * 128

    nc = bacc.Bacc(
        "TRN2", target_bir_lowering=False, debug=False, num_devices=N_CORES
    )
    xT = nc.dram_tensor("xT", [D_IN, t_pad], BF16, kind="ExternalInput").ap()
    w1 = nc.dram_tensor("w1", [D_IN, D_H], BF16, kind="ExternalInput").ap()
    # w2 packed on host: [:, 0:128] = W2[0:128,:], [:, 128:256] = W2[128:256,:]
    w2 = nc.dram_tensor("w2", [128, 2 * D_OUT], BF16, kind="ExternalInput").ap()
    b1 = nc.dram_tensor("b1", [128, 2], F32, kind="ExternalInput").ap()
    b2 = nc.dram_tensor("b2", [128, 1], F32, kind="ExternalInput").ap()
    gidx = nc.dram_tensor("gidx", [128, n_sub * idxp], I16, kind="ExternalInput").ap()
    out = nc.dram_tensor("out", [128, g_len], F32, kind="ExternalOutput").ap()
    # per-sub-window totals: the host adds W_{s-1} to each block-first
    # segment (local scans lose the cross-boundary carry)
    wout = nc.dram_tensor("wout", [128, n_sub], F32, kind="ExternalOutput").ap()

    eng_busy = {"dve": 0.0, "act": 0.0}

    with tile.TileContext(nc) as tc, ExitStack() as ctx:
        singles = ctx.enter_context(tc.tile_pool(name="singles", bufs=1))
        xin = ctx.enter_context(tc.tile_pool(name="xin", bufs=SBUF_BUFS))
        h1sb = ctx.enter_context(tc.tile_pool(name="h1sb", bufs=SBUF_BUFS))
        h2winp = ctx.enter_context(tc.tile_pool(name="h2win", bufs=3))
        winp = ctx.enter_context(tc.tile_pool(name="winp", bufs=3))
        gp = ctx.enter_context(tc.tile_pool(name="gp", bufs=1))
        outp = ctx.enter_context(tc.tile_pool(name="outp", bufs=2))
        # one shared psum ring: h1a/h1b/h2 pair-tiles [128,1024] rotate
        # through 4 slots x 2 banks = all 8 banks; a single tag makes the
        # ring shared so every evacuation is a 1024-element op
        psA = ctx.enter_context(tc.tile_pool(name="psA", bufs=4, space="PSUM"))

        # constants ride the scalar-engine DMA queue so the first input
        # window can issue immediately on the sync queue
        w1s = singles.tile([128, D_H], BF16)
        nc.scalar.dma_start(out=w1s[:], in_=w1[:])
        w2s = singles.tile([128, 2 * D_OUT], BF16)
        nc.scalar.dma_start(out=w2s[:], in_=w2[:])
        b1s = singles.tile([128, 2], F32)
        nc.scalar.dma_start(out=b1s[:], in_=b1[:])
        b2s = singles.tile([128, 1], F32)
        nc.scalar.dma_start(out=b2s[:], in_=b2[:])
        gis = singles.tile([128, n_sub * idxp], I16)
        nc.scalar.dma_start(out=gis[:], in_=gidx[:])
        ones = singles.tile([128, SUB], BF16)
        nc.vector.memset(ones[:], 1.0)
        zcol = singles.tile([128, 1], F32)
        nc.vector.memset(zcol[:], 0.0)

        gpt = gp.tile([128, 1 + g_len], F32, tag="gpad")
        nc.gpsimd.memset(gpt[:], 0.0)
        # touch the activation table at t=0 so the 1.3us table load hides
        # under the first input DMA instead of stalling the first evac
        actwarm = singles.tile([128, 1], F32)
        nc.scalar.activation(actwarm[:], ones[:, 0:2].bitcast(F32), RELU, bias=0.0)

        def evac(dst, src, bias_ap, free):
            """relu(src + bias) -> dst on the less-busy of DVE/ACT."""
            e = min(("dve", "act"), key=lambda k: eng_busy[k] + _COST[(k, free)])
            eng_busy[e] += _COST[(e, free)]
            if e == "act":
                nc.scalar.activation(dst, src, RELU, bias=bias_ap)
            else:
                nc.vector.tensor_scalar(
                    out=dst, in0=src, scalar1=bias_ap, scalar2=0.0, op0=ADD, op1=MAX
                )

        def emit_gather(s, win_s):
            nc.gpsimd.ap_gather(
                out_ap=gpt[:, 1 + s * spw : 1 + (s + 1) * spw],
                in_ap=win_s[:],
                idxs_ap=gis[:, s * idxp : s * idxp + spw16],
                channels=128,
                num_elems=SUB2 + 1,
                d=1,
                num_idxs=spw,
            )
            # export this sub-window's total for the host-side carry fix
            nc.sync.dma_start(
                out=wout[:, s : s + 1], in_=win_s[:, SUB2 : SUB2 + 1]
            )

        def emit_diff(lo, hi):
            """totals[lo:hi] = gpt[1+lo:1+hi] - gpt[lo:hi], then add the
            previous sub-window's total at each sub-window-boundary slot
            (scans are sub-window-local, so cross-boundary diffs lose the
            carry), then DMA out via the Pool queue (the sync queue carries
            the input stream; a diff-gated store would block it)."""
            n = hi - lo
            tt = outp.tile([128, 1024], F32, tag="tot")
            nc.vector.tensor_tensor(
                out=tt[:, 0:n], in0=gpt[:, 1 + lo : 1 + hi],
                in1=gpt[:, lo:hi], op=SUBT,
            )
            eng_busy["dve"] += (58 + n) * 1.04
            nc.gpsimd.dma_start(out=out[:, lo:hi], in_=tt[:, 0:n])

        for _rep in range(reps):
          # timing-only outer repetition; each rep rewrites the same output
          st = {"diffed": 0, "gathered_s": -1}
          pend = []  # [(s, win_tile)] scanned sub-windows, gather deferred
          h2q = []  # [(s, pc_tile)] pairs whose h2 evac is deferred
          p2q = []  # [(s, p2_tile)] pair-sums whose scan is deferred

          def emit_scan(s, p2t):
              # sub-window-LOCAL cumsum over pair-sums (initial = const 0):
              # local scans are mutually independent -- no carry chain; the
              # missing carry is restored on the host from wout. Deferred
              # one pair behind its producer so the DVE never runs the scan
              # directly after the pair-add that feeds it (pipe drain).
              win = winp.tile([128, 1 + SUB2], F32, tag="win", name="win")
              if s == 0:
                  nc.vector.memset(win[:, 0:1], 0.0)
              nc.vector.tensor_tensor_scan(
                  out=win[:, 1 : 1 + SUB2],
                  data0=ones[:, 0:SUB2],
                  data1=p2t[:],
                  initial=zcol[:],
                  op0=MULT,
                  op1=ADD,
              )
              eng_busy["dve"] += 1660.0
              if mode != "scan":
                  pend.append((s, win))

          def flush_h2():
              if not h2q:
                  return
              s, pc = h2q.pop(0)
              if mode == "full" and pend:
                  # gather for the sub-window scanned one flush ago
                  gs, gwin = pend.pop(0)
                  emit_gather(gs, gwin)
                  st["gathered_s"] = gs
                  safe = st["gathered_s"] * spw
                  while safe - st["diffed"] >= 1024:
                      emit_diff(st["diffed"], st["diffed"] + 1024)
                      st["diffed"] += 1024
              h2w = h2winp.tile([128, SUB], BF16, tag="h2w", name="h2w")
              # pinned to ACT: if this evac ran on DVE, the pair-add right
              # after would read its output back-to-back on the same
              # engine and pay the full pipeline drain
              nc.scalar.activation(h2w[:], pc[:], RELU, bias=b2s[:, 0:1])
              eng_busy["act"] += _COST[("act", 1024)]
              # adjacent-pair sums (segments are even-length, so pairs never
              # straddle a boundary): a plain pipelineable add, which halves
              # the length of the scan -- the scan's loop-carried recurrence
              # runs at only ~1 elem / 3 cycles on hardware
              p2t = winp.tile([128, SUB2], F32, tag="p2", name="p2")
              nc.vector.tensor_tensor(
                  out=p2t[:], in0=h2w[:, 0:SUB:2], in1=h2w[:, 1:SUB:2], op=ADD
              )
              eng_busy["dve"] += 660.0
              emit_scan(s, p2t)

          # full DMA windows plus an optional SUB-sized tail window
          win_ofs = list(range(0, t_pad - WIN + 1, WIN))
          if t_pad % WIN:
              win_ofs.append(t_pad - SUB)
          for w, ofs in enumerate(win_ofs):
            wlen = WIN if ofs + WIN <= t_pad else SUB
            # one big input DMA per window (4KB per partition in bf16);
            # window 0 is split per-ITER so the pipeline ramps sooner
            xw = xin.tile([128, WIN], BF16, tag="xw")
            if w == 0:
                for j in range(wlen // ITER):
                    nc.sync.dma_start(
                        out=xw[:, j * ITER : (j + 1) * ITER],
                        in_=xT[:, ofs + j * ITER : ofs + (j + 1) * ITER],
                    )
            else:
                nc.sync.dma_start(
                    out=xw[:, 0:wlen], in_=xT[:, ofs : ofs + wlen]
                )
            if mode == "dma":
                nc.vector.tensor_copy(out=gpt[:, 0:1], in_=xw[:, 0:2].bitcast(F32))
                continue
            for p2 in range(wlen // SUB):
                base = p2 * SUB
                s = (ofs + base) // SUB  # sub-window index
                xh = (xw[:, base : base + ITER], xw[:, base + ITER : base + SUB])
                pa = psA.tile([128, SUB], F32, tag="ps", name="h1a_ps")
                pb = psA.tile([128, SUB], F32, tag="ps", name="h1b_ps")
                # L1: one stationary load per weight half per 1024 tokens
                nc.tensor.matmul(pa[:, 0:ITER], w1s[:, 0:128], xh[0],
                                 start=True, stop=True)
                nc.tensor.matmul(pa[:, ITER:SUB], w1s[:, 0:128], xh[1],
                                 start=True, stop=True)
                nc.tensor.matmul(pb[:, 0:ITER], w1s[:, 128:256], xh[0],
                                 start=True, stop=True)
                nc.tensor.matmul(pb[:, ITER:SUB], w1s[:, 128:256], xh[1],
                                 start=True, stop=True)
                if mode == "mm":
                    nc.vector.tensor_copy(out=gpt[:, 0:1], in_=pa[:, 0:1])
                    nc.vector.tensor_copy(out=gpt[:, 0:1], in_=pb[:, 0:1])
                    continue
                h1a = h1sb.tile([128, SUB], BF16, tag="h1a")
                h1b = h1sb.tile([128, SUB], BF16, tag="h1b")
                # W2a matmuls are ordered right after the h1a evacuation so
                # the PE doesn't also wait on h1b's evacuation
                evac(h1a[:], pa[:], b1s[:, 0:1], 1024)
                pc = psA.tile([128, SUB], F32, tag="ps", name="h2_ps")
                nc.tensor.matmul(pc[:, 0:ITER], w2s[:, 0:128], h1a[:, 0:ITER],
                                 start=True, stop=False)
                nc.tensor.matmul(pc[:, ITER:SUB], w2s[:, 0:128], h1a[:, ITER:SUB],
                                 start=True, stop=False)
                evac(h1b[:], pb[:], b1s[:, 1:2], 1024)
                nc.tensor.matmul(pc[:, 0:ITER], w2s[:, 128:256], h1b[:, 0:ITER],
                                 start=False, stop=True)
                nc.tensor.matmul(pc[:, ITER:SUB], w2s[:, 128:256], h1b[:, ITER:SUB],
                                 start=False, stop=True)
                if mode == "mlp":
                    nc.vector.tensor_copy(out=gpt[:, 0:1], in_=pc[:, 0:1])
                    continue
                h2q.append((s, pc))
                flush_h2()
          if mode in ("full", "scan"):
            flush_h2()
            while p2q:
                emit_scan(*p2q.pop(0))
          if mode == "full":
            total_slots = n_sub * spw
            # diffs that depend only on already-emitted gathers go first so
            # only the last spw slots wait on the final gather
            while pend:
                safe = pend[0][0] * spw
                while safe - st["diffed"] >= 1 and st["diffed"] < safe:
                    take = min(1024, safe - st["diffed"])
                    emit_diff(st["diffed"], st["diffed"] + take)
                    st["diffed"] += take
                gs, gwin = pend.pop(0)
                emit_gather(gs, gwin)
            while st["diffed"] < total_slots:
                take = min(1024, total_slots - st["diffed"])
                emit_diff(st["diffed"], st["diffed"] + take)
                st["diffed"] += take

    nc.compile()
    return nc


def _prepare(x, segment_ids, num_segments):
    """Host-side sharding + gather-index construction. Returns per-core
    metadata and the program size parameters."""
    T_total = x.shape[0]
    n_seg = int(num_segments)
    seg = np.asarray(segment_ids).astype(np.int64)
    counts = np.bincount(seg, minlength=n_seg).astype(np.int64)
    # pad every segment to an even token count (zero tokens, corrected on
    # the host) so adjacent-pair sums never straddle a segment boundary
    counts2 = ((counts + 1) // 2) * 2
    pad = counts2 - counts
    # local scans + single-carry boundary fixup assume a segment never
    # spans more than two sub-windows
    assert counts2.max() < SUB, "segment longer than scan sub-window"
    cum = np.cumsum(counts)
    cum2 = np.cumsum(counts2)

    # whole-segment split balanced by token count
    split = [0]
    for c in range(1, N_CORES):
        target = c * T_total / N_CORES
        s = int(np.searchsorted(cum, target))
        if s + 1 < n_seg and abs(cum[s] - target) < abs(
            (cum[s - 1] if s > 0 else 0) - target
        ):
            s = s + 1
        s = max(split[-1], min(s, n_seg))
        split.append(s)
    split.append(n_seg)

    cores = []
    max_tok = 1
    for c in range(N_CORES):
        s0, s1 = split[c], split[c + 1]
        t0 = int(cum[s0 - 1]) if s0 > 0 else 0
        t1 = int(cum[s1 - 1]) if s1 > 0 else 0
        t0p = int(cum2[s0 - 1]) if s0 > 0 else 0
        t1p = int(cum2[s1 - 1]) if s1 > 0 else 0
        pad_loc = pad[s0:s1]
        # dst column (in the padded stream) of each real token
        pads_before = np.concatenate([[0], np.cumsum(pad_loc[:-1])]) \
            if s1 > s0 else np.zeros(0, dtype=np.int64)
        dst_idx = np.arange(t1 - t0) + np.repeat(pads_before, counts[s0:s1])
        cores.append(
            {"s0": s0, "s1": s1, "t0": t0, "t1": t1, "t0p": t0p, "t1p": t1p,
             "pad": pad_loc, "dst_idx": dst_idx}
        )
        max_tok = max(max_tok, t1p - t0p)

    # pad to SUB granularity (not WIN): the DMA loop handles a SUB-sized
    # tail window, and a whole mostly-pad sub-window is avoided
    t_pad = int(math.ceil(max_tok / SUB) * SUB)
    n_sub = t_pad // SUB

    # per-core per-sub-window segment-end indices (in the padded stream;
    # gather indices count token PAIRS)
    max_ends = 1
    for core in cores:
        s0, s1, t0p = core["s0"], core["s1"], core["t0p"]
        ends = cum2[s0:s1] - 1 - t0p  # local end col per segment; may be -1
        sub_of = np.maximum(ends, 0) // SUB
        idx_rel = (ends - sub_of * SUB + 1) // 2  # pair idx in [0, SUB2]
        core["sub_of"] = sub_of
        core["idx_rel"] = idx_rel
        if len(ends):
            bc = np.bincount(sub_of, minlength=n_sub)
            max_ends = max(max_ends, int(bc.max()))

    spw = int(math.ceil(max_ends / 16) * 16)
    n_tr = int(math.ceil(n_sub * spw / 128))

    for core in cores:
        s0, s1 = core["s0"], core["s1"]
        n_loc = s1 - s0
        slot_of = np.zeros(n_loc, dtype=np.int64)
        idx_full = np.zeros(n_sub * spw, dtype=np.int16)
        pos = np.zeros(n_sub, dtype=np.int64)
        # fill sub-window-by-sub-window in segment order
        for j in range(n_loc):
            w = int(core["sub_of"][j])
            k = int(pos[w])
            assert k < spw
            idx_full[w * spw + k] = core["idx_rel"][j]
            slot_of[j] = w * spw + k
            pos[w] = k + 1
        # pad each sub-window by repeating its last real index (0 if none)
        for w in range(n_sub):
            k = int(pos[w])
            last = idx_full[w * spw + k - 1] if k > 0 else np.int16(0)
            idx_full[w * spw + k : (w + 1) * spw] = last
        core["slot_of"] = slot_of
        # wrap for ap_gather: unwrapped[j] = idxs[j % 16, j // 16] per
        # sub-window, each block padded to a 16B-aligned width
        idxp = ((spw // 16 + 7) // 8) * 8
        blocks = []
        for w in range(n_sub):
            arr = idx_full[w * spw : (w + 1) * spw]
            blk = np.zeros((16, idxp), dtype=np.int16)
            blk[:, : spw // 16] = arr.reshape(spw // 16, 16).T
            blocks.append(blk)
        gidx16 = np.concatenate(blocks, axis=1)  # [16, n_sub * idxp]
        core["gidx"] = np.tile(gidx16, (8, 1)).astype(np.int16)  # [128, ...]
        core["inv"] = (1.0 / np.maximum(counts[s0:s1], 1)).astype(np.float32)

    return cores, t_pad, spw, n_tr


_PROGRAM_CACHE = {}


def _bf16(a):
    import ml_dtypes

    return np.asarray(a, dtype=np.float32).astype(ml_dtypes.bfloat16)


def _make_in_maps(cores, t_pad, x, W1, b1, W2, b2):
    w1_np = _bf16(W1)
    w2_np = _bf16(np.concatenate([W2[:128, :], W2[128:, :]], axis=1))
    b1_np = np.ascontiguousarray(np.stack([b1[:128], b1[128:]], axis=1))
    b2_np = np.ascontiguousarray(b2[:, None])
    in_maps = []
    for core in cores:
        t0, t1 = core["t0"], core["t1"]
        xT_c = np.zeros((D_IN, t_pad), dtype=np.float32)
        xT_c[:, core["dst_idx"]] = x[t0:t1].T
        in_maps.append(
            {
                "xT": _bf16(xT_c),
                "w1": w1_np,
                "w2": w2_np,
                "b1": b1_np,
                "b2": b2_np,
                "gidx": core["gidx"],
            }
        )
    return in_maps


def kernel(x, segment_ids, num_segments, W1, b1, W2, b2):
    x = np.ascontiguousarray(np.asarray(x, dtype=np.float32))
    W1 = np.asarray(W1, dtype=np.float32)
    b1 = np.asarray(b1, dtype=np.float32)
    W2 = np.asarray(W2, dtype=np.float32)
    b2 = np.asarray(b2, dtype=np.float32)
    n_seg = int(num_segments)

    cores, t_pad, spw, n_tr = _prepare(x, segment_ids, num_segments)

    key = (t_pad, spw, n_tr)
    if key not in _PROGRAM_CACHE:
        _PROGRAM_CACHE[key] = _build_program(t_pad, spw, n_tr)
    nc = _PROGRAM_CACHE[key]

    in_maps = _make_in_maps(cores, t_pad, x, W1, b1, W2, b2)

    res = run_bass_kernel_spmd(nc, in_maps, list(range(N_CORES)))

    # constant h2 contribution of a zero pad token, in device arithmetic
    import ml_dtypes

    h1p = _bf16(np.maximum(b1, 0)).astype(np.float32)
    w2b = _bf16(W2).astype(np.float32)
    cpad = _bf16(np.maximum(h1p @ w2b + b2, 0)).astype(np.float32)  # [128]

    out_full = np.zeros((n_seg, D_OUT), dtype=np.float32)
    for c, core in enumerate(cores):
        s0, s1 = core["s0"], core["s1"]
        if s1 <= s0:
            continue
        slot = core["slot_of"]
        vals = res.results[c]["out"][:, slot]  # [128, n_loc] segment sums
        # cross-sub-window carry: block-first segments add the previous
        # sub-window's total (scans are sub-window-local)
        wout = res.results[c]["wout"]  # [128, n_sub]
        first = (slot % spw == 0) & (slot >= spw)
        if first.any():
            vals[:, first] += wout[:, slot[first] // spw - 1]
        # remove the pad tokens' constant contribution, then mean
        vals = vals - cpad[:, None] * core["pad"][None, :]
        out_full[s0:s1] = (vals * core["inv"][None, :]).T
    return out_full


# revision 71
# speedup vs baseline: 1.5011x; 1.0255x over previous
"""DeepSets (MLP + ragged segment-mean) Trainium2 Bass kernel.

Full inputs in / full outputs out. Internally: data-parallel over sets --
tokens are sharded by contiguous whole-segment ranges across 8 NeuronCores
(balanced by token count), the tiny MLP weights are replicated, and the
segment-mean is fully local per core.

Design (per-core), driven by measured TRN2 hardware behavior:
  - x and weights in bf16: halves the dominant HBM stream (matmul rate on
    TRN2 is 1 col/cycle for both bf16 and fp32r, so only DMA gains).
  - L1/L2 matmuls feature-major (weights stationary), fp32 PSUM; all three
    PSUM streams (h1a/h1b/h2) rotate through one shared 4-slot x 2-bank
    ring so every evacuation is a [128,1024] op.
  - PSUM evacuations (bias+relu, cast to bf16) run on DVE + ACT only
    (the Pool/GPSIMD engine cannot touch PSUM and its ISA has no tensor
    ALU ops), placed by a greedy static load balancer.
  - Segment-mean via per-sub-window (1024-token) machinery:
      * segments are padded to EVEN length with zero tokens on the host
        (+~1.6% tokens; their constant MLP contribution is subtracted on
        the host), so adjacent-token PAIR SUMS never straddle a segment;
      * a cheap pipelineable strided add folds token pairs, then a
        tensor_tensor_scan over only 512 pair-sums builds the local
        cumsum -- the scan's loop-carried recurrence runs at ~3 cycles
        per element on real hardware, so halving its length matters, and
        sub-window-LOCAL scans (constant zero initial) avoid the carried
        scan->scan dependency chain whose per-dependent-op pipeline
        drains dominated earlier versions;
      * GpSimd ap_gather picks the cumsum at host-computed segment-end
        pair indices, deferred one sub-window to avoid queue stalls;
      * adjacent diff of the gathered values gives segment sums; each
        sub-window total is DMA'd out and the host adds the missing
        cross-boundary carry to block-first segments.
  - Output leaves the device feature-major [128, slots]; the host does the
    transpose and the 1/count scaling (no on-device transpose/scale).
"""

import math
from contextlib import ExitStack

import numpy as np

import concourse.bass as bass
import concourse.tile as tile
from concourse import bacc, mybir
from concourse.bass_utils import run_bass_kernel_spmd

N_CORES = 8
D_IN, D_H, D_OUT = 128, 256, 128
WIN = 2048  # tokens per input-DMA window
SUB = 1024  # tokens per scan/gather sub-window (= one h2 evac pair)
SUB2 = SUB // 2  # pair-sums per sub-window (segments are padded to even
#                  length so every segment boundary falls between pairs)
ITER = 512  # tokens per MLP pipeline iteration (= one fp32 psum bank)
SBUF_BUFS = 3

F32 = mybir.dt.float32
BF16 = mybir.dt.bfloat16
I16 = mybir.dt.int16
RELU = mybir.ActivationFunctionType.Relu
ADD = mybir.AluOpType.add
SUBT = mybir.AluOpType.subtract
MULT = mybir.AluOpType.mult
MAX = mybir.AluOpType.max

# static-schedule costs (ns) for psum evacs by engine and free size,
# calibrated against TimelineSim engine-busy traces
_COST = {
    ("dve", 512): 700.0,
    ("act", 512): 615.0,
    ("dve", 1024): 1260.0,
    ("act", 1024): 1070.0,
}


def _build_program(t_pad: int, spw: int, n_tr: int, reps: int = 1, mode: str = "full"):
    """Build the single-core SPMD program for t_pad tokens per core.

    spw: gather slots per window (multiple of 16)
    n_tr: number of 128-slot output tiles (out cols = n_tr*128)
    reps: execute the whole pipeline this many times (timing use only)
    mode: "full" | "dma" | "mm" | "mlp" | "scan" -- ablation timing only
    """
    n_sub = t_pad // SUB
    spw16 = spw // 16
    idxp = ((spw16 + 7) // 8) * 8
    g_len = n_tr * 128

    nc = bacc.Bacc(
        "TRN2", target_bir_lowering=False, debug=False, num_devices=N_CORES
    )
    xT = nc.dram_tensor("xT", [D_IN, t_pad], BF16, kind="ExternalInput").ap()
    w1 = nc.dram_tensor("w1", [D_IN, D_H], BF16, kind="ExternalInput").ap()
    # w2 packed on host: [:, 0:128] = W2[0:128,:], [:, 128:256] = W2[128:256,:]
    w2 = nc.dram_tensor("w2", [128, 2 * D_OUT], BF16, kind="ExternalInput").ap()
    b1 = nc.dram_tensor("b1", [128, 2], F32, kind="ExternalInput").ap()
    b2 = nc.dram_tensor("b2", [128, 1], F32, kind="ExternalInput").ap()
    gidx = nc.dram_tensor("gidx", [128, n_sub * idxp], I16, kind="ExternalInput").ap()
    out = nc.dram_tensor("out", [128, g_len], F32, kind="ExternalOutput").ap()
    # per-sub-window totals: the host adds W_{s-1} to each block-first
    # segment (local scans lose the cross-boundary carry)
    wout = nc.dram_tensor("wout", [128, n_sub], F32, kind="ExternalOutput").ap()

    eng_busy = {"dve": 0.0, "act": 0.0}

    with tile.TileContext(nc) as tc, ExitStack() as ctx:
        singles = ctx.enter_context(tc.tile_pool(name="singles", bufs=1))
        xin = ctx.enter_context(tc.tile_pool(name="xin", bufs=SBUF_BUFS))
        h1sb = ctx.enter_context(tc.tile_pool(name="h1sb", bufs=SBUF_BUFS))
        h2winp = ctx.enter_context(tc.tile_pool(name="h2win", bufs=3))
        winp = ctx.enter_context(tc.tile_pool(name="winp", bufs=3))
        gp = ctx.enter_context(tc.tile_pool(name="gp", bufs=1))
        outp = ctx.enter_context(tc.tile_pool(name="outp", bufs=2))
        # one shared psum ring: h1a/h1b/h2 pair-tiles [128,1024] rotate
        # through 4 slots x 2 banks = all 8 banks; a single tag makes the
        # ring shared so every evacuation is a 1024-element op
        psA = ctx.enter_context(tc.tile_pool(name="psA", bufs=4, space="PSUM"))

        # constants ride the scalar-engine DMA queue so the first input
        # window can issue immediately on the sync queue
        w1s = singles.tile([128, D_H], BF16)
        nc.scalar.dma_start(out=w1s[:], in_=w1[:])
        w2s = singles.tile([128, 2 * D_OUT], BF16)
        nc.scalar.dma_start(out=w2s[:], in_=w2[:])
        b1s = singles.tile([128, 2], F32)
        nc.scalar.dma_start(out=b1s[:], in_=b1[:])
        b2s = singles.tile([128, 1], F32)
        nc.scalar.dma_start(out=b2s[:], in_=b2[:])
        gis = singles.tile([128, n_sub * idxp], I16)
        nc.scalar.dma_start(out=gis[:], in_=gidx[:])
        ones = singles.tile([128, SUB], BF16)
        nc.vector.memset(ones[:], 1.0)
        zcol = singles.tile([128, 1], F32)
        nc.vector.memset(zcol[:], 0.0)

        gpt = gp.tile([128, 1 + g_len], F32, tag="gpad")
        nc.gpsimd.memset(gpt[:], 0.0)
        # touch the activation table at t=0 so the 1.3us table load hides
        # under the first input DMA instead of stalling the first evac
        actwarm = singles.tile([128, 1], F32)
        nc.scalar.activation(actwarm[:], ones[:, 0:2].bitcast(F32), RELU, bias=0.0)

        def evac(dst, src, bias_ap, free):
            """relu(src + bias) -> dst on the less-busy of DVE/ACT."""
            e = min(("dve", "act"), key=lambda k: eng_busy[k] + _COST[(k, free)])
            eng_busy[e] += _COST[(e, free)]
            if e == "act":
                nc.scalar.activation(dst, src, RELU, bias=bias_ap)
            else:
                nc.vector.tensor_scalar(
                    out=dst, in0=src, scalar1=bias_ap, scalar2=0.0, op0=ADD, op1=MAX
                )

        def emit_gather(s, win_s):
            nc.gpsimd.ap_gather(
                out_ap=gpt[:, 1 + s * spw : 1 + (s + 1) * spw],
                in_ap=win_s[:],
                idxs_ap=gis[:, s * idxp : s * idxp + spw16],
                channels=128,
                num_elems=SUB2 + 1,
                d=1,
                num_idxs=spw,
            )
            # export this sub-window's total for the host-side carry fix
            nc.sync.dma_start(
                out=wout[:, s : s + 1], in_=win_s[:, SUB2 : SUB2 + 1]
            )

        def emit_diff(lo, hi):
            """totals[lo:hi] = gpt[1+lo:1+hi] - gpt[lo:hi], then add the
            previous sub-window's total at each sub-window-boundary slot
            (scans are sub-window-local, so cross-boundary diffs lose the
            carry), then DMA out via the Pool queue (the sync queue carries
            the input stream; a diff-gated store would block it)."""
            n = hi - lo
            tt = outp.tile([128, 1024], F32, tag="tot")
            nc.vector.tensor_tensor(
                out=tt[:, 0:n], in0=gpt[:, 1 + lo : 1 + hi],
                in1=gpt[:, lo:hi], op=SUBT,
            )
            eng_busy["dve"] += (58 + n) * 1.04
            nc.gpsimd.dma_start(out=out[:, lo:hi], in_=tt[:, 0:n])

        for _rep in range(reps):
          # timing-only outer repetition; each rep rewrites the same output
          st = {"diffed": 0, "gathered_s": -1}
          pend = []  # [(s, win_tile)] scanned sub-windows, gather deferred
          h2q = []  # [(s, pc_tile)] pairs whose h2 evac is deferred
          p2q = []  # [(s, p2_tile)] pair-sums whose scan is deferred

          def emit_scan(s, p2t):
              # sub-window-LOCAL cumsum over pair-sums (initial = const 0):
              # local scans are mutually independent -- no carry chain; the
              # missing carry is restored on the host from wout. Deferred
              # one pair behind its producer so the DVE never runs the scan
              # directly after the pair-add that feeds it (pipe drain).
              win = winp.tile([128, 1 + SUB2], F32, tag="win", name="win")
              if s == 0:
                  nc.vector.memset(win[:, 0:1], 0.0)
              nc.vector.tensor_tensor_scan(
                  out=win[:, 1 : 1 + SUB2],
                  data0=ones[:, 0:SUB2],
                  data1=p2t[:],
                  initial=zcol[:],
                  op0=MULT,
                  op1=ADD,
              )
              eng_busy["dve"] += 1660.0
              if mode != "scan":
                  pend.append((s, win))

          def flush_h2():
              if not h2q:
                  return
              s, pc = h2q.pop(0)
              if mode == "full" and pend:
                  # gather for the sub-window scanned one flush ago
                  gs, gwin = pend.pop(0)
                  emit_gather(gs, gwin)
                  st["gathered_s"] = gs
                  safe = st["gathered_s"] * spw
                  while safe - st["diffed"] >= 1024:
                      emit_diff(st["diffed"], st["diffed"] + 1024)
                      st["diffed"] += 1024
              h2w = h2winp.tile([128, SUB], BF16, tag="h2w", name="h2w")
              # pinned to ACT: if this evac ran on DVE, the pair-add right
              # after would read its output back-to-back on the same
              # engine and pay the full pipeline drain. The output access
              # pattern DEINTERLEAVES even/odd tokens into two dense
              # halves (free for ACT) so the pair-add below is a dense
              # all-bf16 tensor_tensor, eligible for the DVE 2x_1P mode.
              nc.scalar.activation(
                  h2w[:].rearrange("p (j t) -> p j t", j=2),
                  pc[:].rearrange("p (t j) -> p j t", j=2),
                  RELU,
                  bias=b2s[:, 0:1],
              )
              eng_busy["act"] += _COST[("act", 1024)]
              # adjacent-pair sums (segments are even-length, so pairs never
              # straddle a boundary): a pipelineable 2x-mode add, which
              # halves the length of the scan -- the scan's loop-carried
              # recurrence runs at only ~1 elem / 3 cycles on hardware
              p2t = winp.tile([128, SUB2], BF16, tag="p2", name="p2")
              nc.vector.tensor_tensor(
                  out=p2t[:], in0=h2w[:, 0:SUB2], in1=h2w[:, SUB2:SUB], op=ADD
              )
              eng_busy["dve"] += 400.0
              emit_scan(s, p2t)

          # full DMA windows plus an optional SUB-sized tail window
          win_ofs = list(range(0, t_pad - WIN + 1, WIN))
          if t_pad % WIN:
              win_ofs.append(t_pad - SUB)
          for w, ofs in enumerate(win_ofs):
            wlen = WIN if ofs + WIN <= t_pad else SUB
            # one big input DMA per window (4KB per partition in bf16);
            # window 0 is split per-ITER so the pipeline ramps sooner
            xw = xin.tile([128, WIN], BF16, tag="xw")
            if w == 0:
                for j in range(wlen // ITER):
                    nc.sync.dma_start(
                        out=xw[:, j * ITER : (j + 1) * ITER],
                        in_=xT[:, ofs + j * ITER : ofs + (j + 1) * ITER],
                    )
            else:
                nc.sync.dma_start(
                    out=xw[:, 0:wlen], in_=xT[:, ofs : ofs + wlen]
                )
            if mode == "dma":
                nc.vector.tensor_copy(out=gpt[:, 0:1], in_=xw[:, 0:2].bitcast(F32))
                continue
            for p2 in range(wlen // SUB):
                base = p2 * SUB
                s = (ofs + base) // SUB  # sub-window index
                xh = (xw[:, base : base + ITER], xw[:, base + ITER : base + SUB])
                pa = psA.tile([128, SUB], F32, tag="ps", name="h1a_ps")
                pb = psA.tile([128, SUB], F32, tag="ps", name="h1b_ps")
                # L1: one stationary load per weight half per 1024 tokens
                nc.tensor.matmul(pa[:, 0:ITER], w1s[:, 0:128], xh[0],
                                 start=True, stop=True)
                nc.tensor.matmul(pa[:, ITER:SUB], w1s[:, 0:128], xh[1],
                                 start=True, stop=True)
                nc.tensor.matmul(pb[:, 0:ITER], w1s[:, 128:256], xh[0],
                                 start=True, stop=True)
                nc.tensor.matmul(pb[:, ITER:SUB], w1s[:, 128:256], xh[1],
                                 start=True, stop=True)
                if mode == "mm":
                    nc.vector.tensor_copy(out=gpt[:, 0:1], in_=pa[:, 0:1])
                    nc.vector.tensor_copy(out=gpt[:, 0:1], in_=pb[:, 0:1])
                    continue
                h1a = h1sb.tile([128, SUB], BF16, tag="h1a")
                h1b = h1sb.tile([128, SUB], BF16, tag="h1b")
                # W2a matmuls are ordered right after the h1a evacuation so
                # the PE doesn't also wait on h1b's evacuation
                evac(h1a[:], pa[:], b1s[:, 0:1], 1024)
                pc = psA.tile([128, SUB], F32, tag="ps", name="h2_ps")
                nc.tensor.matmul(pc[:, 0:ITER], w2s[:, 0:128], h1a[:, 0:ITER],
                                 start=True, stop=False)
                nc.tensor.matmul(pc[:, ITER:SUB], w2s[:, 0:128], h1a[:, ITER:SUB],
                                 start=True, stop=False)
                evac(h1b[:], pb[:], b1s[:, 1:2], 1024)
                nc.tensor.matmul(pc[:, 0:ITER], w2s[:, 128:256], h1b[:, 0:ITER],
                                 start=False, stop=True)
                nc.tensor.matmul(pc[:, ITER:SUB], w2s[:, 128:256], h1b[:, ITER:SUB],
                                 start=False, stop=True)
                if mode == "mlp":
                    nc.vector.tensor_copy(out=gpt[:, 0:1], in_=pc[:, 0:1])
                    continue
                h2q.append((s, pc))
                flush_h2()
          if mode in ("full", "scan"):
            flush_h2()
            while p2q:
                emit_scan(*p2q.pop(0))
          if mode == "full":
            total_slots = n_sub * spw
            # diffs that depend only on already-emitted gathers go first so
            # only the last spw slots wait on the final gather
            while pend:
                safe = pend[0][0] * spw
                while safe - st["diffed"] >= 1 and st["diffed"] < safe:
                    take = min(1024, safe - st["diffed"])
                    emit_diff(st["diffed"], st["diffed"] + take)
                    st["diffed"] += take
                gs, gwin = pend.pop(0)
                emit_gather(gs, gwin)
            while st["diffed"] < total_slots:
                take = min(1024, total_slots - st["diffed"])
                emit_diff(st["diffed"], st["diffed"] + take)
                st["diffed"] += take

    nc.compile()
    return nc


def _prepare(x, segment_ids, num_segments):
    """Host-side sharding + gather-index construction. Returns per-core
    metadata and the program size parameters."""
    T_total = x.shape[0]
    n_seg = int(num_segments)
    seg = np.asarray(segment_ids).astype(np.int64)
    counts = np.bincount(seg, minlength=n_seg).astype(np.int64)
    # pad every segment to an even token count (zero tokens, corrected on
    # the host) so adjacent-pair sums never straddle a segment boundary
    counts2 = ((counts + 1) // 2) * 2
    pad = counts2 - counts
    # local scans + single-carry boundary fixup assume a segment never
    # spans more than two sub-windows
    assert counts2.max() < SUB, "segment longer than scan sub-window"
    cum = np.cumsum(counts)
    cum2 = np.cumsum(counts2)

    # whole-segment split balanced by token count
    split = [0]
    for c in range(1, N_CORES):
        target = c * T_total / N_CORES
        s = int(np.searchsorted(cum, target))
        if s + 1 < n_seg and abs(cum[s] - target) < abs(
            (cum[s - 1] if s > 0 else 0) - target
        ):
            s = s + 1
        s = max(split[-1], min(s, n_seg))
        split.append(s)
    split.append(n_seg)

    cores = []
    max_tok = 1
    for c in range(N_CORES):
        s0, s1 = split[c], split[c + 1]
        t0 = int(cum[s0 - 1]) if s0 > 0 else 0
        t1 = int(cum[s1 - 1]) if s1 > 0 else 0
        t0p = int(cum2[s0 - 1]) if s0 > 0 else 0
        t1p = int(cum2[s1 - 1]) if s1 > 0 else 0
        pad_loc = pad[s0:s1]
        # dst column (in the padded stream) of each real token
        pads_before = np.concatenate([[0], np.cumsum(pad_loc[:-1])]) \
            if s1 > s0 else np.zeros(0, dtype=np.int64)
        dst_idx = np.arange(t1 - t0) + np.repeat(pads_before, counts[s0:s1])
        cores.append(
            {"s0": s0, "s1": s1, "t0": t0, "t1": t1, "t0p": t0p, "t1p": t1p,
             "pad": pad_loc, "dst_idx": dst_idx}
        )
        max_tok = max(max_tok, t1p - t0p)

    # pad to SUB granularity (not WIN): the DMA loop handles a SUB-sized
    # tail window, and a whole mostly-pad sub-window is avoided
    t_pad = int(math.ceil(max_tok / SUB) * SUB)
    n_sub = t_pad // SUB

    # per-core per-sub-window segment-end indices (in the padded stream;
    # gather indices count token PAIRS)
    max_ends = 1
    for core in cores:
        s0, s1, t0p = core["s0"], core["s1"], core["t0p"]
        ends = cum2[s0:s1] - 1 - t0p  # local end col per segment; may be -1
        sub_of = np.maximum(ends, 0) // SUB
        idx_rel = (ends - sub_of * SUB + 1) // 2  # pair idx in [0, SUB2]
        core["sub_of"] = sub_of
        core["idx_rel"] = idx_rel
        if len(ends):
            bc = np.bincount(sub_of, minlength=n_sub)
            max_ends = max(max_ends, int(bc.max()))

    spw = int(math.ceil(max_ends / 16) * 16)
    n_tr = int(math.ceil(n_sub * spw / 128))

    for core in cores:
        s0, s1 = core["s0"], core["s1"]
        n_loc = s1 - s0
        slot_of = np.zeros(n_loc, dtype=np.int64)
        idx_full = np.zeros(n_sub * spw, dtype=np.int16)
        pos = np.zeros(n_sub, dtype=np.int64)
        # fill sub-window-by-sub-window in segment order
        for j in range(n_loc):
            w = int(core["sub_of"][j])
            k = int(pos[w])
            assert k < spw
            idx_full[w * spw + k] = core["idx_rel"][j]
            slot_of[j] = w * spw + k
            pos[w] = k + 1
        # pad each sub-window by repeating its last real index (0 if none)
        for w in range(n_sub):
            k = int(pos[w])
            last = idx_full[w * spw + k - 1] if k > 0 else np.int16(0)
            idx_full[w * spw + k : (w + 1) * spw] = last
        core["slot_of"] = slot_of
        # wrap for ap_gather: unwrapped[j] = idxs[j % 16, j // 16] per
        # sub-window, each block padded to a 16B-aligned width
        idxp = ((spw // 16 + 7) // 8) * 8
        blocks = []
        for w in range(n_sub):
            arr = idx_full[w * spw : (w + 1) * spw]
            blk = np.zeros((16, idxp), dtype=np.int16)
            blk[:, : spw // 16] = arr.reshape(spw // 16, 16).T
            blocks.append(blk)
        gidx16 = np.concatenate(blocks, axis=1)  # [16, n_sub * idxp]
        core["gidx"] = np.tile(gidx16, (8, 1)).astype(np.int16)  # [128, ...]
        core["inv"] = (1.0 / np.maximum(counts[s0:s1], 1)).astype(np.float32)

    return cores, t_pad, spw, n_tr


_PROGRAM_CACHE = {}


def _bf16(a):
    import ml_dtypes

    return np.asarray(a, dtype=np.float32).astype(ml_dtypes.bfloat16)


def _make_in_maps(cores, t_pad, x, W1, b1, W2, b2):
    w1_np = _bf16(W1)
    w2_np = _bf16(np.concatenate([W2[:128, :], W2[128:, :]], axis=1))
    b1_np = np.ascontiguousarray(np.stack([b1[:128], b1[128:]], axis=1))
    b2_np = np.ascontiguousarray(b2[:, None])
    in_maps = []
    for core in cores:
        t0, t1 = core["t0"], core["t1"]
        xT_c = np.zeros((D_IN, t_pad), dtype=np.float32)
        xT_c[:, core["dst_idx"]] = x[t0:t1].T
        in_maps.append(
            {
                "xT": _bf16(xT_c),
                "w1": w1_np,
                "w2": w2_np,
                "b1": b1_np,
                "b2": b2_np,
                "gidx": core["gidx"],
            }
        )
    return in_maps


def kernel(x, segment_ids, num_segments, W1, b1, W2, b2):
    x = np.ascontiguousarray(np.asarray(x, dtype=np.float32))
    W1 = np.asarray(W1, dtype=np.float32)
    b1 = np.asarray(b1, dtype=np.float32)
    W2 = np.asarray(W2, dtype=np.float32)
    b2 = np.asarray(b2, dtype=np.float32)
    n_seg = int(num_segments)

    cores, t_pad, spw, n_tr = _prepare(x, segment_ids, num_segments)

    key = (t_pad, spw, n_tr)
    if key not in _PROGRAM_CACHE:
        _PROGRAM_CACHE[key] = _build_program(t_pad, spw, n_tr)
    nc = _PROGRAM_CACHE[key]

    in_maps = _make_in_maps(cores, t_pad, x, W1, b1, W2, b2)

    res = run_bass_kernel_spmd(nc, in_maps, list(range(N_CORES)))

    # constant h2 contribution of a zero pad token, in device arithmetic
    import ml_dtypes

    h1p = _bf16(np.maximum(b1, 0)).astype(np.float32)
    w2b = _bf16(W2).astype(np.float32)
    cpad = _bf16(np.maximum(h1p @ w2b + b2, 0)).astype(np.float32)  # [128]

    out_full = np.zeros((n_seg, D_OUT), dtype=np.float32)
    for c, core in enumerate(cores):
        s0, s1 = core["s0"], core["s1"]
        if s1 <= s0:
            continue
        slot = core["slot_of"]
        vals = res.results[c]["out"][:, slot]  # [128, n_loc] segment sums
        # cross-sub-window carry: block-first segments add the previous
        # sub-window's total (scans are sub-window-local)
        wout = res.results[c]["wout"]  # [128, n_sub]
        first = (slot % spw == 0) & (slot >= spw)
        if first.any():
            vals[:, first] += wout[:, slot[first] // spw - 1]
        # remove the pad tokens' constant contribution, then mean
        vals = vals - cpad[:, None] * core["pad"][None, :]
        out_full[s0:s1] = (vals * core["inv"][None, :]).T
    return out_full
